# revision 43
# baseline (speedup 1.0000x reference)
"""GATv2 (2-layer) Trainium2 Bass kernel, 8-core SPMD, single fused NEFF.

Wall-clock-oriented design (device exec is ~0.1s; build/compile/transfer
dominate):
- ONE kernel for both layers; h is exchanged on-device with an AllGather
  collective (no inter-layer host round trip, one compile, one launch).
- Uniform node sharding: 784 blocks of 128 nodes, 98 blocks per core, so
  AllGather slices concatenate into global node order and one edge-index
  stream serves both layers.
- xl/xr tables are row-interleaved ([2N, C]: row 2n = xl_n, 2n+1 = xr_n),
  so gathers use indices 2*src and 2*dst+1 into the same table.
- All per-core inputs are packed into a single f32 blob (one sharded
  transfer); int32 stream regions are viewed via AP.bitcast.
- Edge phase: per 128-edge tile only 5 instructions (2 gathers, one-hot
  build, exp-prescale into an rhs buffer that also carries the exp column,
  and ONE aggregation matmul over [cout+H] columns); the logit pipeline is
  batched over CH=32 tiles with broadcast APs.
- Segment softmax without max subtraction (logits are O(1); exact enough),
  denominator applied after aggregation. leaky_relu via 0.6x + 0.4|x|.
- Final output in bf16 (value-proportional rounding keeps relative error
  safe); inputs/tables stay f32 (bf16 there creates absolute-scale errors
  that blow up the relative-error metric at near-zero outputs).
"""

import json
import os
import sys
import threading
import time as _time
import numpy as np

# Smaller/faster NEFF packaging (no debug info); read by walrus arg builder.
os.environ.setdefault("CONCOURSE_SCRUB_NEFF_DEBUG_INFO", "1")

_T0 = _time.time()


def _mark(msg):
    print(f"[kernel +{_time.time() - _T0:6.2f}s] {msg}", file=sys.stderr, flush=True)

import concourse.bass as bass
import concourse.mybir as mybir
from concourse.tile import TileContext, ScopedClock
from concourse.bass_utils import run_bass_kernel_spmd
from concourse.masks import make_identity

# ----------------------------------------------------------------------------
# Workarounds for the walrus build in this container: at most ONE sync-wait
# per instruction. Extra waits are peeled onto NoOps inserted just before.
# ----------------------------------------------------------------------------
_MAXW = 1
_split_counter = [0]


def _patched_drain_and_barrier(self, tick_clock, wait_clock):
    d0 = self.nc.sync.drain()
    wait_clock.add_sem_waits(d0.ins, ScopedClock({None: tick_clock.global_clock}))
    waits = list(d0.ins.sync_info.on_wait)
    if len(waits) > _MAXW:
        del d0.ins.sync_info.on_wait[_MAXW:]
        rest = waits[_MAXW:]
        for i in range(0, len(rest), _MAXW):
            d = self.nc.sync.drain()
            if d.ins.sync_info is None:
                d.ins.sync_info = mybir.SyncInfo(on_update=[], on_wait=[])
            d.ins.sync_info.on_wait.extend(rest[i:i + _MAXW])
    self.nc.all_engine_barrier()
    popped = self.nc._tile_sem_poison_stack.pop()
    assert popped is self._sem_poison
    self.nc.clear_and_free_semaphores(list(self.sems.allocated().values()))
    self.nc.all_engine_barrier()


def _fix_bir_json(data: bytes) -> bytes:
    try:
        import orjson
        _loads, _dumps = orjson.loads, lambda m: orjson.dumps(m)
    except ImportError:
        _loads, _dumps = json.loads, lambda m: json.dumps(m).encode()
    m = _loads(data)
    changed = False
    for f in m.get("functions", []):
        for b in f.get("blocks", []):
            insts = b.get("instructions")
            if not insts:
                continue
            out = []
            for inst in insts:
                si = inst.get("sync_info") or {}
                waits = si.get("on_wait") or []
                if len(waits) > 1:
                    for w in waits[:-1]:
                        _split_counter[0] += 1
                        out.append({
                            "name": f"I-sw{_split_counter[0]}",
                            "opcode": "NoOp",
                            "engine": inst.get("engine"),
                            "ins": [], "outs": [],
                            "sync_info": {"on_update": [], "on_wait": [w]},
                        })
                    si["on_wait"] = [waits[-1]]
                    changed = True
                out.append(inst)
            b["instructions"] = out
    if not changed:
        return data
    return _dumps(m)


def _install_fixes():
    TileContext._drain_and_barrier = _patched_drain_and_barrier
    if not getattr(bass.Bass, "_tilefix_json", False):
        orig = bass.Bass.to_json_bytes

        def to_json_bytes(self, *a, **k):
            return _fix_bir_json(orig(self, *a, **k))

        bass.Bass.to_json_bytes = to_json_bytes
        bass.Bass._tilefix_json = True


_install_fixes()


def _install_fast_walrus():
    """Skip the birverifier pass (validation-only; this BIR is known-valid)
    to cut client-side compile time."""
    import concourse.bass_utils as bu
    from pathlib import Path
    from concourse.aot_env import aot_getenv

    if getattr(bu, "_fast_walrus", False):
        return

    def fast_bvo(tmpdir, inp="bir.json", outp="file.neff", arch=None, *,
                 dve_root=None):
        cmd = [
            bu.get_walrus_driver(),
            "--pass",
            ",".join(["runtime_memory_reservation", "lower_act", "lower_dve",
                      "lower_ap_offset", "codegen", "neff_packager"]),
            "-i", inp,
            "--neff-output-filename", outp,
            "--enable-birsim=true",
            "--mem-mode=physical",
            "--policy=0",
            "--enable-ldw-opt=false",
            "--assign-static-dmas-to-sp=false",
            f"--dram-page-size={aot_getenv('NEURON_SCRATCHPAD_PAGE_SIZE', '256')}",
            "--enable-neff-debug-info=false",
            "--jobs", "8",
            *bu.get_walrus_args(
                bu.get_bir_arch(tmpdir, inp) if arch is None else arch,
                tmpdir, dve_root=dve_root),
        ]
        result = bu.run_command(cmd, cwd=tmpdir)
        if result is not None:
            (Path(tmpdir) / "log.txt").write_text(result.stdout)
        return f"{tmpdir}/{outp}"

    bu.bir_verify_and_optimise = fast_bvo
    bu._fast_walrus = True


_install_fast_walrus()

# ----------------------------------------------------------------------------
N_NODES = 100_000
N_EDGES = 1_600_000
F_IN = 128
H1, C1 = 2, 64
H2, C2 = 1, 64
CO1, CO2 = H1 * C1, H2 * C2            # 128, 64
NCORES = 8
P = 128
NBLKC = 98                              # blocks per core
NLOC = NBLKC * P                        # 12544 nodes per core
NTOT = NCORES * NLOC                    # 100352 padded nodes
CH = 32                                 # tiles per merged logit chunk
F32 = mybir.dt.float32
BF16 = mybir.dt.bfloat16
I32 = mybir.dt.int32
AL = mybir.AluOpType
AF = mybir.ActivationFunctionType


def _rep(v):
    v = np.asarray(v, np.float32).reshape(1, -1)
    return np.ascontiguousarray(np.repeat(v, P, axis=0))


def _prep_edges(edge_index, edge_attr):
    """Sort edges by dst; build per-core [128, Tpad] streams (vectorized)."""
    src = np.asarray(edge_index[0], np.int64)
    dst = np.asarray(edge_index[1], np.int64)
    E = src.shape[0]
    order = np.argsort(dst, kind="stable")
    src_s = src[order].astype(np.int64)
    dst_s = dst[order].astype(np.int64)
    ea_s = np.asarray(edge_attr, np.float32).reshape(-1)[order]
    blk = (dst_s >> 7).astype(np.int64)            # global block 0..781
    cnt = np.bincount(blk, minlength=NCORES * NBLKC)
    T_slot = np.maximum((cnt.reshape(NCORES, NBLKC) + P - 1) // P, 1).max(axis=0)
    col0 = np.zeros(NBLKC + 1, np.int64)
    col0[1:] = np.cumsum(T_slot)
    sumT = int(col0[-1])
    Tpad = ((sumT + CH - 1) // CH) * CH
    T_slot = T_slot.copy()
    T_slot[-1] += Tpad - sumT                      # tail pad columns absorb
    runstart = np.zeros(NCORES * NBLKC + 1, np.int64)
    runstart[1:] = np.cumsum(cnt)
    rank = np.arange(E, dtype=np.int64) - runstart[blk]
    core = blk // NBLKC
    slot = blk - core * NBLKC
    col = col0[slot] + (rank >> 7)
    row = rank & 127

    idx_st = np.zeros((NCORES, P, Tpad), np.int32)
    dst_st = np.ones((NCORES, P, Tpad), np.int32)
    dr_st = np.full((NCORES, P, Tpad), -1.0, np.float32)
    ea_st = np.zeros((NCORES, P, Tpad), np.float32)
    idx_st[core, row, col] = (2 * src_s).astype(np.int32)
    dst_st[core, row, col] = (2 * dst_s + 1).astype(np.int32)
    dr_st[core, row, col] = (dst_s & 127).astype(np.float32)
    ea_st[core, row, col] = ea_s
    return dict(Tpad=Tpad, T_slot=T_slot.astype(np.int64), idx_st=idx_st,
                dst_st=dst_st, dr_st=dr_st, ea_st=ea_st)


def _build_kernel(Tpad, T_slot):
    nc = bass.Bass()

    # ---- blob layout (element offsets into the per-core [1, NW] f32 blob)
    widths = dict(iotaV=P, W1l=CO1, W1r=CO1, blr1=2 * CO1, vV1=CO1,
                  attV1=CO1, b1=CO1, W2l=CO2, W2r=CO2, blr2=2 * CO2,
                  vV2=CO2, attV2=CO2, b2=CO2)
    offs = {}
    off = 0
    for k, w in widths.items():
        offs[k] = off
        off += P * w
    offs["xT"] = off
    off += P * NLOC
    for s in ("idx", "dstg", "dr", "ea"):
        offs[s] = off
        off += P * Tpad
    NW = off

    blob = nc.dram_tensor("blob", [1, NW], F32, kind="ExternalInput")
    out_d = nc.dram_tensor("out", [NLOC, CO2], BF16, kind="ExternalOutput")
    XLR1_loc = nc.dram_tensor("XLR1_loc", [2 * NLOC, CO1], F32)
    XLR1 = nc.dram_tensor("XLR1", [2 * NTOT, CO1], F32)
    HL1 = nc.dram_tensor("HL1", [NLOC, CO1], F32)
    XLR2_loc = nc.dram_tensor("XLR2_loc", [2 * NLOC, CO2], F32)
    XLR2 = nc.dram_tensor("XLR2", [2 * NTOT, CO2], F32)

    def ap2d(name, w=None):
        o, tw = offs[name], widths.get(name, Tpad if name in
                                       ("idx", "dstg", "dr", "ea") else None)
        if name == "xT":
            tw = NLOC
        if w is None:
            w = tw
        return blob[0:1, o:o + P * tw].rearrange("o (p w) -> (o p) w", p=P)

    with TileContext(nc) as tc:
        with (
            tc.tile_pool(name="const", bufs=1) as cp,
            tc.tile_pool(name="dense", bufs=3) as dp,
            tc.tile_pool(name="st", bufs=3) as sp,
            tc.tile_pool(name="chunk", bufs=2) as chp,
            tc.tile_pool(name="tile", bufs=6) as tp,
            tc.tile_pool(name="ep", bufs=2) as epp,
            tc.tile_pool(name="pd", bufs=2, space="PSUM") as ppd,
            tc.tile_pool(name="po", bufs=2, space="PSUM") as ppo,
            tc.tile_pool(name="pt", bufs=2, space="PSUM") as ppt,
        ):
            C = {}
            for k, w in widths.items():
                t = cp.tile([P, w], F32, tag=f"c_{k}")
                nc.sync.dma_start(out=t[:], in_=ap2d(k))
                C[k] = t
            ident = cp.tile([P, P], F32)
            make_identity(nc, ident[:])
            Szero = cp.tile([P, P], F32)
            nc.vector.tensor_scalar(out=Szero[:], in0=ident[:], scalar1=0.0,
                                    scalar2=None, op0=AL.mult)

            def dense(xsrc_ap_of_blk, Wl, Wr, blr, dst_dram, cout, transpose_in):
                for j in range(NBLKC):
                    if transpose_in:
                        ht = dp.tile([P, P], F32, tag="ht")
                        nc.sync.dma_start(out=ht[:], in_=xsrc_ap_of_blk(j))
                        pT = ppt.tile([P, P], F32, space="PSUM")
                        nc.tensor.transpose(out=pT[:], in_=ht[:],
                                            identity=ident[:])
                        xt = dp.tile([P, P], F32, tag="xt")
                        nc.scalar.copy(xt[:], pT[:])
                    else:
                        xt = dp.tile([P, P], F32, tag="xt")
                        nc.sync.dma_start(out=xt[:], in_=xsrc_ap_of_blk(j))
                    ps = ppd.tile([P, 2 * cout], F32, space="PSUM")
                    nc.tensor.matmul(ps[:, 0:cout], lhsT=xt[:], rhs=Wl[:],
                                     start=True, stop=True)
                    nc.tensor.matmul(ps[:, cout:2 * cout], lhsT=xt[:], rhs=Wr[:],
                                     start=True, stop=True)
                    xlr = dp.tile([P, 2 * cout], F32, tag="xlr")
                    nc.vector.tensor_tensor(out=xlr[:], in0=ps[:], in1=blr[:],
                                            op=AL.add)
                    oap = dst_dram[j * 2 * P:(j + 1) * 2 * P, :].rearrange(
                        "(p two) c -> p (two c)", two=2)
                    nc.sync.dma_start(out=oap, in_=xlr[:])

            def edge_phase(table, cout, H, vV, attV, biasV, out_dram, relu,
                           out_dt=F32):
                Cc = cout // H
                # block bookkeeping per global column
                blk_of, start_c, stop_c = [], [], []
                for s in range(NBLKC):
                    for t in range(int(T_slot[s])):
                        blk_of.append(s)
                        start_c.append(t == 0)
                        stop_c.append(t == int(T_slot[s]) - 1)
                psO = None
                for g in range(Tpad // CH):
                    idxc = sp.tile([P, CH], I32, tag="idxc")
                    nc.sync.dma_start(out=idxc[:], in_=ap2d("idx")[:, g * CH:(g + 1) * CH].bitcast(I32))
                    dstc = sp.tile([P, CH], I32, tag="dstc")
                    nc.sync.dma_start(out=dstc[:], in_=ap2d("dstg")[:, g * CH:(g + 1) * CH].bitcast(I32))
                    drc = sp.tile([P, CH], F32, tag="drc")
                    nc.sync.dma_start(out=drc[:], in_=ap2d("dr")[:, g * CH:(g + 1) * CH])
                    eac = sp.tile([P, CH], F32, tag="eac")
                    nc.sync.dma_start(out=eac[:], in_=ap2d("ea")[:, g * CH:(g + 1) * CH])

                    W = cout + H          # rhs row: [scaled msg | ex]
                    # allocate at layer-1 sizes so L2 reuses the same slots;
                    # only the first CH*cout (resp. CH*W) columns are used.
                    msgA_t = chp.tile([P, CH * CO1], F32, tag="msgA")
                    m_t = chp.tile([P, CH * CO1], F32, tag="m")
                    wk_t = chp.tile([P, CH * CO1], F32, tag="wk")
                    rhs_t = chp.tile([P, CH * (CO1 + H1)], F32, tag="rhs")
                    tabs_t = chp.tile([P, CH * CO1], F32, tag="tabs")
                    msgA = msgA_t[:, 0:CH * cout]
                    m = m_t[:, 0:CH * cout]
                    wk = wk_t[:, 0:CH * cout]
                    rhs = rhs_t[:, 0:CH * W]
                    tabs = tabs_t[:, 0:CH * cout]
                    for t in range(CH):
                        nc.gpsimd.indirect_dma_start(
                            out=msgA[:, t * cout:(t + 1) * cout], out_offset=None,
                            in_=table[:, :],
                            in_offset=bass.IndirectOffsetOnAxis(ap=idxc[:, t:t + 1], axis=0))
                        nc.gpsimd.indirect_dma_start(
                            out=m[:, t * cout:(t + 1) * cout], out_offset=None,
                            in_=table[:, :],
                            in_offset=bass.IndirectOffsetOnAxis(ap=dstc[:, t:t + 1], axis=0))
                    # m = msgA + xr[dst] ; m += ea * vV (broadcast)
                    nc.vector.tensor_tensor(out=m[:], in0=m[:], in1=msgA[:], op=AL.add)
                    eb = eac[:].rearrange("p (t o) -> p t o", o=1)
                    vb = vV[:].rearrange("p (o c) -> p o c", o=1)
                    ebb, vbb = bass.broadcast_tensor_aps(eb, vb)
                    mv = m[:].rearrange("p (t c) -> p t c", t=CH)
                    wkv = wk[:].rearrange("p (t c) -> p t c", t=CH)
                    nc.vector.tensor_tensor(out=wkv, in0=ebb, in1=vbb, op=AL.mult)
                    nc.vector.tensor_tensor(out=m[:], in0=m[:], in1=wk[:], op=AL.add)
                    # tabs = |m| ; q = m*att ; lin = reduce ; u = |m|*att ; ur
                    nc.scalar.activation(tabs[:], m[:], AF.Abs)
                    av = attV[:].rearrange("p (o c) -> p o c", o=1)
                    _, avb = bass.broadcast_tensor_aps(mv, av)
                    nc.vector.tensor_tensor(out=wkv, in0=mv, in1=avb, op=AL.mult)
                    lin = sp.tile([P, CH * H], F32, tag="lin")
                    nc.vector.tensor_reduce(out=lin[:],
                                            in_=wk[:].rearrange("p (th c) -> p th c", c=Cc),
                                            axis=mybir.AxisListType.X, op=AL.add)
                    tv = tabs[:].rearrange("p (t c) -> p t c", t=CH)
                    nc.vector.tensor_tensor(out=wkv, in0=tv, in1=avb, op=AL.mult)
                    ur = sp.tile([P, CH * H], F32, tag="ur")
                    nc.vector.tensor_reduce(out=ur[:],
                                            in_=wk[:].rearrange("p (th c) -> p th c", c=Cc),
                                            axis=mybir.AxisListType.X, op=AL.add)
                    logit = sp.tile([P, CH * H], F32, tag="logit")
                    nc.vector.tensor_scalar(out=logit[:], in0=lin[:], scalar1=0.6,
                                            scalar2=None, op0=AL.mult)
                    nc.vector.scalar_tensor_tensor(out=logit[:], in0=ur[:], scalar=0.4,
                                                   in1=logit[:], op0=AL.mult, op1=AL.add)
                    ex = sp.tile([P, CH * H], F32, tag="ex")
                    nc.scalar.activation(ex[:], logit[:], AF.Exp)
                    # copy ex into the tail H columns of each tile's rhs slot
                    exdst = rhs[:].rearrange("p (t w) -> p t w", w=W)[:, :, cout:cout + H]
                    nc.scalar.copy(exdst, ex[:].rearrange("p (t h) -> p t h", h=H))

                    for t in range(CH):
                        c = g * CH + t
                        s = blk_of[c]
                        if start_c[c]:
                            psO = ppo.tile([P, W], F32, space="PSUM")
                            # the first start=True accumulation is dropped by
                            # HW; absorb it with a zero matmul per block.
                            nc.tensor.matmul(psO[:], lhsT=Szero[:],
                                             rhs=C["blr1"][:, 0:W],
                                             start=True, stop=False)
                        S01 = tp.tile([P, P], F32, tag="S01")
                        nc.vector.tensor_scalar(out=S01[:], in0=C["iotaV"][:],
                                                scalar1=drc[:, t:t + 1], scalar2=None,
                                                op0=AL.is_equal)
                        # scaled = msgA_tile * ex (per-head broadcast) -> rhs slot
                        sc = rhs[:, t * W:t * W + cout].rearrange(
                            "p (h c) -> p h c", h=H)
                        mg = msgA[:, t * cout:(t + 1) * cout].rearrange(
                            "p (h c) -> p h c", h=H)
                        eview = ex[:, t * H:(t + 1) * H].rearrange("p (h o) -> p h o", o=1)
                        _, evb = bass.broadcast_tensor_aps(mg, eview)
                        nc.vector.tensor_tensor(out=sc, in0=mg, in1=evb, op=AL.mult)
                        nc.tensor.matmul(psO[:], lhsT=S01[:],
                                         rhs=rhs[:, t * W:(t + 1) * W],
                                         start=False, stop=bool(stop_c[c]))
                        if stop_c[c]:
                            den = epp.tile([P, H], F32, tag="den")
                            nc.vector.tensor_scalar_max(den[:], psO[:, cout:cout + H], 1e-30)
                            dinv = epp.tile([P, H], F32, tag="dinv")
                            nc.vector.reciprocal(dinv[:], den[:])
                            hsb = epp.tile([P, cout], F32, tag="hsb")
                            hv = hsb[:].rearrange("p (h c) -> p h c", h=H)
                            pv = psO[:, 0:cout].rearrange("p (h c) -> p h c", h=H)
                            dv = dinv[:].rearrange("p (h o) -> p h o", o=1)
                            _, dvb = bass.broadcast_tensor_aps(pv, dv)
                            nc.vector.tensor_tensor(out=hv, in0=pv, in1=dvb, op=AL.mult)
                            hfin = epp.tile([P, cout], out_dt, tag="hfin")
                            nc.vector.tensor_tensor(out=hfin[:], in0=hsb[:], in1=biasV[:],
                                                    op=AL.add)
                            if relu:
                                nc.vector.tensor_scalar_max(hfin[:], hfin[:], 0.0)
                            nc.sync.dma_start(out=out_dram[s * P:(s + 1) * P, :],
                                              in_=hfin[:])

            # ---------- layer 1 ----------
            dense(lambda j: ap2d("xT")[:, j * P:(j + 1) * P],
                  C["W1l"], C["W1r"], C["blr1"], XLR1_loc, CO1, transpose_in=False)
            nc.gpsimd.collective_compute(
                "AllGather", AL.bypass, replica_groups=[list(range(NCORES))],
                ins=[XLR1_loc[:, :]], outs=[XLR1[:, :]])
            edge_phase(XLR1, CO1, H1, C["vV1"], C["attV1"], C["b1"], HL1, relu=True)
            # ---------- layer 2 ----------
            dense(lambda j: HL1[j * P:(j + 1) * P, :],
                  C["W2l"], C["W2r"], C["blr2"], XLR2_loc, CO2, transpose_in=True)
            nc.gpsimd.collective_compute(
                "AllGather", AL.bypass, replica_groups=[list(range(NCORES))],
                ins=[XLR2_loc[:, :]], outs=[XLR2[:, :]])
            edge_phase(XLR2, CO2, H2, C["vV2"], C["attV2"], C["b2"], out_d,
                       relu=False, out_dt=BF16)
    return nc, offs, NW


def _make_consts(W1_l, b1_l, W1_r, b1_r, W1_e, att1, bias1,
                 W2_l, b2_l, W2_r, b2_r, W2_e, att2, bias2):
    iota = np.repeat(np.arange(P, dtype=np.float32)[None, :], P, axis=0)
    return [
        iota,
        np.asarray(W1_l, np.float32), np.asarray(W1_r, np.float32),
        _rep(np.concatenate([np.asarray(b1_l).ravel(), np.asarray(b1_r).ravel()])),
        _rep(np.asarray(W1_e).ravel()), _rep(np.asarray(att1).ravel()),
        _rep(np.asarray(bias1).ravel()),
        np.asarray(W2_l, np.float32), np.asarray(W2_r, np.float32),
        _rep(np.concatenate([np.asarray(b2_l).ravel(), np.asarray(b2_r).ravel()])),
        _rep(np.asarray(W2_e).ravel()), _rep(np.asarray(att2).ravel()),
        _rep(np.asarray(bias2).ravel()),
    ]


def _warm_devices():
    """Establish the axon/PJRT session (network handshakes) while the main
    thread does CPU-bound prep/build; device_put releases the GIL."""
    try:
        import jax
        d = jax.devices()
        jax.device_put(np.zeros((8, 8), np.float32), d[0]).block_until_ready()
    except Exception:
        pass


def kernel(x, edge_index, edge_attr,
           W1_l, b1_l, W1_r, b1_r, W1_e, att1, bias1,
           W2_l, b2_l, W2_r, b2_r, W2_e, att2, bias2):
    _mark("kernel start")
    warm = threading.Thread(target=_warm_devices, daemon=True)
    warm.start()
    x = np.asarray(x, np.float32)
    prep = _prep_edges(edge_index, edge_attr)
    Tpad = prep["Tpad"]
    _mark("prep done")

    nc, offs, NW = _build_kernel(Tpad, prep["T_slot"])
    _mark("build done")

    consts = _make_consts(W1_l, b1_l, W1_r, b1_r, W1_e, att1, bias1,
                          W2_l, b2_l, W2_r, b2_r, W2_e, att2, bias2)
    cvec = np.concatenate([c.ravel() for c in consts]).astype(np.float32)

    xpad = np.zeros((NTOT, F_IN), np.float32)
    xpad[:N_NODES] = x
    xT = np.ascontiguousarray(xpad.T)          # [128, NTOT]

    in_maps = []
    for k in range(NCORES):
        blobv = np.empty((1, NW), np.float32)
        o = 0
        blobv[0, o:o + cvec.size] = cvec
        o += cvec.size
        blobv[0, o:o + P * NLOC] = xT[:, k * NLOC:(k + 1) * NLOC].ravel()
        o += P * NLOC
        for arr in (prep["idx_st"][k].view(np.float32),
                    prep["dst_st"][k].view(np.float32),
                    prep["dr_st"][k], prep["ea_st"][k]):
            blobv[0, o:o + P * Tpad] = arr.ravel()
            o += P * Tpad
        assert o == NW
        in_maps.append({"blob": blobv})
    _mark("blobs packed")
    warm.join(timeout=120)

    res = run_bass_kernel_spmd(nc, in_maps, core_ids=list(range(NCORES)))
    _mark("spmd run done")
    out = np.concatenate([res.results[k]["out"] for k in range(NCORES)], axis=0)
    _mark("fetch done")
    return out[:N_NODES].astype(np.float32)


# revision 44
# speedup vs baseline: 2.0317x; 2.0317x over previous
"""GATv2 (2-layer) Trainium2 Bass kernel, 8-core SPMD, single fused NEFF.

Wall-clock-oriented design (device exec is ~0.1s; build/compile/transfer
dominate):
- ONE kernel for both layers; h is exchanged on-device with an AllGather
  collective (no inter-layer host round trip, one compile, one launch).
- Uniform node sharding: 784 blocks of 128 nodes, 98 blocks per core, so
  AllGather slices concatenate into global node order and one edge-index
  stream serves both layers.
- xl/xr tables are row-interleaved ([2N, C]: row 2n = xl_n, 2n+1 = xr_n),
  so gathers use indices 2*src and 2*dst+1 into the same table.
- All per-core inputs are packed into a single f32 blob (one sharded
  transfer); int32 stream regions are viewed via AP.bitcast.
- Edge phase: per 128-edge tile only 5 instructions (2 gathers, one-hot
  build, exp-prescale into an rhs buffer that also carries the exp column,
  and ONE aggregation matmul over [cout+H] columns); the logit pipeline is
  batched over CH=32 tiles with broadcast APs.
- Segment softmax without max subtraction (logits are O(1); exact enough),
  denominator applied after aggregation. leaky_relu via 0.6x + 0.4|x|.
- Final output in bf16 (value-proportional rounding keeps relative error
  safe); inputs/tables stay f32 (bf16 there creates absolute-scale errors
  that blow up the relative-error metric at near-zero outputs).
"""

import json
import os
import sys
import threading
import time as _time
import numpy as np

# Smaller/faster NEFF packaging (no debug info); read by walrus arg builder.
os.environ.setdefault("CONCOURSE_SCRUB_NEFF_DEBUG_INFO", "1")

_T0 = _time.time()


def _mark(msg):
    print(f"[kernel +{_time.time() - _T0:6.2f}s] {msg}", file=sys.stderr, flush=True)

import concourse.bass as bass
import concourse.mybir as mybir
from concourse.tile import TileContext, ScopedClock
from concourse.bass_utils import run_bass_kernel_spmd
from concourse.masks import make_identity

# ----------------------------------------------------------------------------
# Workarounds for the walrus build in this container: at most ONE sync-wait
# per instruction. Extra waits are peeled onto NoOps inserted just before.
# ----------------------------------------------------------------------------
_MAXW = 1
_split_counter = [0]


def _patched_drain_and_barrier(self, tick_clock, wait_clock):
    d0 = self.nc.sync.drain()
    wait_clock.add_sem_waits(d0.ins, ScopedClock({None: tick_clock.global_clock}))
    waits = list(d0.ins.sync_info.on_wait)
    if len(waits) > _MAXW:
        del d0.ins.sync_info.on_wait[_MAXW:]
        rest = waits[_MAXW:]
        for i in range(0, len(rest), _MAXW):
            d = self.nc.sync.drain()
            if d.ins.sync_info is None:
                d.ins.sync_info = mybir.SyncInfo(on_update=[], on_wait=[])
            d.ins.sync_info.on_wait.extend(rest[i:i + _MAXW])
    self.nc.all_engine_barrier()
    popped = self.nc._tile_sem_poison_stack.pop()
    assert popped is self._sem_poison
    self.nc.clear_and_free_semaphores(list(self.sems.allocated().values()))
    self.nc.all_engine_barrier()


def _fix_bir_json(data: bytes) -> bytes:
    try:
        import orjson
        _loads, _dumps = orjson.loads, lambda m: orjson.dumps(m)
    except ImportError:
        _loads, _dumps = json.loads, lambda m: json.dumps(m).encode()
    m = _loads(data)
    changed = False
    for f in m.get("functions", []):
        for b in f.get("blocks", []):
            insts = b.get("instructions")
            if not insts:
                continue
            out = []
            for inst in insts:
                si = inst.get("sync_info") or {}
                waits = si.get("on_wait") or []
                if len(waits) > 1:
                    for w in waits[:-1]:
                        _split_counter[0] += 1
                        out.append({
                            "name": f"I-sw{_split_counter[0]}",
                            "opcode": "NoOp",
                            "engine": inst.get("engine"),
                            "ins": [], "outs": [],
                            "sync_info": {"on_update": [], "on_wait": [w]},
                        })
                    si["on_wait"] = [waits[-1]]
                    changed = True
                out.append(inst)
            b["instructions"] = out
    if not changed:
        return data
    return _dumps(m)


def _install_fixes():
    TileContext._drain_and_barrier = _patched_drain_and_barrier
    if not getattr(bass.Bass, "_tilefix_json", False):
        orig = bass.Bass.to_json_bytes

        def to_json_bytes(self, *a, **k):
            return _fix_bir_json(orig(self, *a, **k))

        bass.Bass.to_json_bytes = to_json_bytes
        bass.Bass._tilefix_json = True


_install_fixes()


def _install_fast_walrus():
    """Skip the birverifier pass (validation-only; this BIR is known-valid)
    to cut client-side compile time."""
    import concourse.bass_utils as bu
    from pathlib import Path
    from concourse.aot_env import aot_getenv

    if getattr(bu, "_fast_walrus", False):
        return

    def fast_bvo(tmpdir, inp="bir.json", outp="file.neff", arch=None, *,
                 dve_root=None):
        cmd = [
            bu.get_walrus_driver(),
            "--pass",
            ",".join(["runtime_memory_reservation", "lower_act", "lower_dve",
                      "lower_ap_offset", "codegen", "neff_packager"]),
            "-i", inp,
            "--neff-output-filename", outp,
            "--enable-birsim=true",
            "--mem-mode=physical",
            "--policy=0",
            "--enable-ldw-opt=false",
            "--assign-static-dmas-to-sp=false",
            f"--dram-page-size={aot_getenv('NEURON_SCRATCHPAD_PAGE_SIZE', '256')}",
            "--enable-neff-debug-info=false",
            "--jobs", "8",
            *bu.get_walrus_args(
                bu.get_bir_arch(tmpdir, inp) if arch is None else arch,
                tmpdir, dve_root=dve_root),
        ]
        result = bu.run_command(cmd, cwd=tmpdir)
        if result is not None:
            (Path(tmpdir) / "log.txt").write_text(result.stdout)
        return f"{tmpdir}/{outp}"

    bu.bir_verify_and_optimise = fast_bvo
    bu._fast_walrus = True


_install_fast_walrus()

# ----------------------------------------------------------------------------
N_NODES = 100_000
N_EDGES = 1_600_000
F_IN = 128
H1, C1 = 2, 64
H2, C2 = 1, 64
CO1, CO2 = H1 * C1, H2 * C2            # 128, 64
NCORES = 8
P = 128
NBLKC = 98                              # blocks per core
NLOC = NBLKC * P                        # 12544 nodes per core
NTOT = NCORES * NLOC                    # 100352 padded nodes
CH = 32                                 # tiles per merged logit chunk
F32 = mybir.dt.float32
BF16 = mybir.dt.bfloat16
I32 = mybir.dt.int32
AL = mybir.AluOpType
AF = mybir.ActivationFunctionType


def _rep(v):
    v = np.asarray(v, np.float32).reshape(1, -1)
    return np.ascontiguousarray(np.repeat(v, P, axis=0))


def _prep_edges(edge_index, edge_attr):
    """Sort edges by dst; build per-core [128, Tpad] streams (vectorized)."""
    src = np.asarray(edge_index[0], np.int64)
    dst = np.asarray(edge_index[1], np.int64)
    E = src.shape[0]
    order = np.argsort(dst, kind="stable")
    src_s = src[order].astype(np.int64)
    dst_s = dst[order].astype(np.int64)
    ea_s = np.asarray(edge_attr, np.float32).reshape(-1)[order]
    blk = (dst_s >> 7).astype(np.int64)            # global block 0..781
    cnt = np.bincount(blk, minlength=NCORES * NBLKC)
    T_slot = np.maximum((cnt.reshape(NCORES, NBLKC) + P - 1) // P, 1).max(axis=0)
    col0 = np.zeros(NBLKC + 1, np.int64)
    col0[1:] = np.cumsum(T_slot)
    sumT = int(col0[-1])
    Tpad = ((sumT + CH - 1) // CH) * CH
    T_slot = T_slot.copy()
    T_slot[-1] += Tpad - sumT                      # tail pad columns absorb
    runstart = np.zeros(NCORES * NBLKC + 1, np.int64)
    runstart[1:] = np.cumsum(cnt)
    rank = np.arange(E, dtype=np.int64) - runstart[blk]
    core = blk // NBLKC
    slot = blk - core * NBLKC
    col = col0[slot] + (rank >> 7)
    row = rank & 127

    idx_st = np.zeros((NCORES, P, Tpad), np.int32)
    dst_st = np.ones((NCORES, P, Tpad), np.int32)
    dr_st = np.full((NCORES, P, Tpad), -1.0, np.float32)
    ea_st = np.zeros((NCORES, P, Tpad), np.float32)
    idx_st[core, row, col] = (2 * src_s).astype(np.int32)
    dst_st[core, row, col] = (2 * dst_s + 1).astype(np.int32)
    dr_st[core, row, col] = (dst_s & 127).astype(np.float32)
    ea_st[core, row, col] = ea_s
    return dict(Tpad=Tpad, T_slot=T_slot.astype(np.int64), idx_st=idx_st,
                dst_st=dst_st, dr_st=dr_st, ea_st=ea_st)


def _build_kernel(Tpad, T_slot):
    nc = bass.Bass()

    # ---- blob layout (element offsets into the per-core [1, NW] f32 blob)
    widths = dict(iotaV=P, W1l=CO1, W1r=CO1, blr1=2 * CO1, vV1=CO1,
                  attV1=CO1, b1=CO1, W2l=CO2, W2r=CO2, blr2=2 * CO2,
                  vV2=CO2, attV2=CO2, b2=CO2)
    offs = {}
    off = 0
    for k, w in widths.items():
        offs[k] = off
        off += P * w
    offs["xT"] = off
    off += P * NLOC
    for s in ("idx", "dstg", "dr", "ea"):
        offs[s] = off
        off += P * Tpad
    NW = off

    blob = nc.dram_tensor("blob", [1, NW], F32, kind="ExternalInput")
    out_d = nc.dram_tensor("out", [NLOC, CO2], BF16, kind="ExternalOutput")
    XLR1_loc = nc.dram_tensor("XLR1_loc", [2 * NLOC, CO1], F32)
    XLR1 = nc.dram_tensor("XLR1", [2 * NTOT, CO1], F32)
    HL1 = nc.dram_tensor("HL1", [NLOC, CO1], F32)
    XLR2_loc = nc.dram_tensor("XLR2_loc", [2 * NLOC, CO2], F32)
    XLR2 = nc.dram_tensor("XLR2", [2 * NTOT, CO2], F32)

    def ap2d(name, w=None):
        o, tw = offs[name], widths.get(name, Tpad if name in
                                       ("idx", "dstg", "dr", "ea") else None)
        if name == "xT":
            tw = NLOC
        if w is None:
            w = tw
        return blob[0:1, o:o + P * tw].rearrange("o (p w) -> (o p) w", p=P)

    with TileContext(nc) as tc:
        with (
            tc.tile_pool(name="const", bufs=1) as cp,
            tc.tile_pool(name="dense", bufs=3) as dp,
            tc.tile_pool(name="st", bufs=3) as sp,
            tc.tile_pool(name="chunk", bufs=2) as chp,
            tc.tile_pool(name="tile", bufs=6) as tp,
            tc.tile_pool(name="ep", bufs=2) as epp,
            tc.tile_pool(name="pd", bufs=2, space="PSUM") as ppd,
            tc.tile_pool(name="po", bufs=2, space="PSUM") as ppo,
            tc.tile_pool(name="pt", bufs=2, space="PSUM") as ppt,
        ):
            C = {}
            for k, w in widths.items():
                t = cp.tile([P, w], F32, tag=f"c_{k}")
                nc.sync.dma_start(out=t[:], in_=ap2d(k))
                C[k] = t
            ident = cp.tile([P, P], F32)
            make_identity(nc, ident[:])
            Szero = cp.tile([P, P], F32)
            nc.vector.tensor_scalar(out=Szero[:], in0=ident[:], scalar1=0.0,
                                    scalar2=None, op0=AL.mult)

            def dense(xsrc_ap_of_blk, Wl, Wr, blr, dst_dram, cout, transpose_in):
                for j in range(NBLKC):
                    if transpose_in:
                        ht = dp.tile([P, P], F32, tag="ht")
                        nc.sync.dma_start(out=ht[:], in_=xsrc_ap_of_blk(j))
                        pT = ppt.tile([P, P], F32, space="PSUM")
                        nc.tensor.transpose(out=pT[:], in_=ht[:],
                                            identity=ident[:])
                        xt = dp.tile([P, P], F32, tag="xt")
                        nc.scalar.copy(xt[:], pT[:])
                    else:
                        xt = dp.tile([P, P], F32, tag="xt")
                        nc.sync.dma_start(out=xt[:], in_=xsrc_ap_of_blk(j))
                    ps = ppd.tile([P, 2 * cout], F32, space="PSUM")
                    nc.tensor.matmul(ps[:, 0:cout], lhsT=xt[:], rhs=Wl[:],
                                     start=True, stop=True)
                    nc.tensor.matmul(ps[:, cout:2 * cout], lhsT=xt[:], rhs=Wr[:],
                                     start=True, stop=True)
                    xlr = dp.tile([P, 2 * cout], F32, tag="xlr")
                    nc.vector.tensor_tensor(out=xlr[:], in0=ps[:], in1=blr[:],
                                            op=AL.add)
                    oap = dst_dram[j * 2 * P:(j + 1) * 2 * P, :].rearrange(
                        "(p two) c -> p (two c)", two=2)
                    nc.sync.dma_start(out=oap, in_=xlr[:])

            def edge_phase(table, cout, H, vV, attV, biasV, out_dram, relu,
                           out_dt=F32):
                Cc = cout // H
                # block bookkeeping per global column
                blk_of, start_c, stop_c = [], [], []
                for s in range(NBLKC):
                    for t in range(int(T_slot[s])):
                        blk_of.append(s)
                        start_c.append(t == 0)
                        stop_c.append(t == int(T_slot[s]) - 1)
                psO = None
                for g in range(Tpad // CH):
                    idxc = sp.tile([P, CH], I32, tag="idxc")
                    nc.sync.dma_start(out=idxc[:], in_=ap2d("idx")[:, g * CH:(g + 1) * CH].bitcast(I32))
                    dstc = sp.tile([P, CH], I32, tag="dstc")
                    nc.sync.dma_start(out=dstc[:], in_=ap2d("dstg")[:, g * CH:(g + 1) * CH].bitcast(I32))
                    drc = sp.tile([P, CH], F32, tag="drc")
                    nc.sync.dma_start(out=drc[:], in_=ap2d("dr")[:, g * CH:(g + 1) * CH])
                    eac = sp.tile([P, CH], F32, tag="eac")
                    nc.sync.dma_start(out=eac[:], in_=ap2d("ea")[:, g * CH:(g + 1) * CH])

                    W = cout + H          # rhs row: [scaled msg | ex]
                    # allocate at layer-1 sizes so L2 reuses the same slots;
                    # only the first CH*cout (resp. CH*W) columns are used.
                    msgA_t = chp.tile([P, CH * CO1], F32, tag="msgA")
                    m_t = chp.tile([P, CH * CO1], F32, tag="m")
                    wk_t = chp.tile([P, CH * CO1], F32, tag="wk")
                    rhs_t = chp.tile([P, CH * (CO1 + H1)], F32, tag="rhs")
                    tabs_t = chp.tile([P, CH * CO1], F32, tag="tabs")
                    msgA = msgA_t[:, 0:CH * cout]
                    m = m_t[:, 0:CH * cout]
                    wk = wk_t[:, 0:CH * cout]
                    rhs = rhs_t[:, 0:CH * W]
                    tabs = tabs_t[:, 0:CH * cout]
                    for t in range(CH):
                        nc.gpsimd.indirect_dma_start(
                            out=msgA[:, t * cout:(t + 1) * cout], out_offset=None,
                            in_=table[:, :],
                            in_offset=bass.IndirectOffsetOnAxis(ap=idxc[:, t:t + 1], axis=0))
                        nc.gpsimd.indirect_dma_start(
                            out=m[:, t * cout:(t + 1) * cout], out_offset=None,
                            in_=table[:, :],
                            in_offset=bass.IndirectOffsetOnAxis(ap=dstc[:, t:t + 1], axis=0))
                    # m = msgA + xr[dst] ; m += ea * vV (broadcast)
                    nc.vector.tensor_tensor(out=m[:], in0=m[:], in1=msgA[:], op=AL.add)
                    eb = eac[:].rearrange("p (t o) -> p t o", o=1)
                    vb = vV[:].rearrange("p (o c) -> p o c", o=1)
                    ebb, vbb = bass.broadcast_tensor_aps(eb, vb)
                    mv = m[:].rearrange("p (t c) -> p t c", t=CH)
                    wkv = wk[:].rearrange("p (t c) -> p t c", t=CH)
                    nc.vector.tensor_tensor(out=wkv, in0=ebb, in1=vbb, op=AL.mult)
                    nc.vector.tensor_tensor(out=m[:], in0=m[:], in1=wk[:], op=AL.add)
                    # tabs = |m| ; q = m*att ; lin = reduce ; u = |m|*att ; ur
                    nc.scalar.activation(tabs[:], m[:], AF.Abs)
                    av = attV[:].rearrange("p (o c) -> p o c", o=1)
                    _, avb = bass.broadcast_tensor_aps(mv, av)
                    nc.vector.tensor_tensor(out=wkv, in0=mv, in1=avb, op=AL.mult)
                    lin = sp.tile([P, CH * H], F32, tag="lin")
                    nc.vector.tensor_reduce(out=lin[:],
                                            in_=wk[:].rearrange("p (th c) -> p th c", c=Cc),
                                            axis=mybir.AxisListType.X, op=AL.add)
                    tv = tabs[:].rearrange("p (t c) -> p t c", t=CH)
                    nc.vector.tensor_tensor(out=wkv, in0=tv, in1=avb, op=AL.mult)
                    ur = sp.tile([P, CH * H], F32, tag="ur")
                    nc.vector.tensor_reduce(out=ur[:],
                                            in_=wk[:].rearrange("p (th c) -> p th c", c=Cc),
                                            axis=mybir.AxisListType.X, op=AL.add)
                    logit = sp.tile([P, CH * H], F32, tag="logit")
                    nc.vector.tensor_scalar(out=logit[:], in0=lin[:], scalar1=0.6,
                                            scalar2=None, op0=AL.mult)
                    nc.vector.scalar_tensor_tensor(out=logit[:], in0=ur[:], scalar=0.4,
                                                   in1=logit[:], op0=AL.mult, op1=AL.add)
                    ex = sp.tile([P, CH * H], F32, tag="ex")
                    nc.scalar.activation(ex[:], logit[:], AF.Exp)
                    # copy ex into the tail H columns of each tile's rhs slot
                    exdst = rhs[:].rearrange("p (t w) -> p t w", w=W)[:, :, cout:cout + H]
                    nc.scalar.copy(exdst, ex[:].rearrange("p (t h) -> p t h", h=H))

                    for t in range(CH):
                        c = g * CH + t
                        s = blk_of[c]
                        if start_c[c]:
                            psO = ppo.tile([P, W], F32, space="PSUM")
                            # the first start=True accumulation is dropped by
                            # HW; absorb it with a zero matmul per block.
                            nc.tensor.matmul(psO[:], lhsT=Szero[:],
                                             rhs=C["blr1"][:, 0:W],
                                             start=True, stop=False)
                        S01 = tp.tile([P, P], F32, tag="S01")
                        nc.vector.tensor_scalar(out=S01[:], in0=C["iotaV"][:],
                                                scalar1=drc[:, t:t + 1], scalar2=None,
                                                op0=AL.is_equal)
                        # scaled = msgA_tile * ex (per-head broadcast) -> rhs slot
                        sc = rhs[:, t * W:t * W + cout].rearrange(
                            "p (h c) -> p h c", h=H)
                        mg = msgA[:, t * cout:(t + 1) * cout].rearrange(
                            "p (h c) -> p h c", h=H)
                        eview = ex[:, t * H:(t + 1) * H].rearrange("p (h o) -> p h o", o=1)
                        _, evb = bass.broadcast_tensor_aps(mg, eview)
                        nc.vector.tensor_tensor(out=sc, in0=mg, in1=evb, op=AL.mult)
                        nc.tensor.matmul(psO[:], lhsT=S01[:],
                                         rhs=rhs[:, t * W:(t + 1) * W],
                                         start=False, stop=bool(stop_c[c]))
                        if stop_c[c]:
                            den = epp.tile([P, H], F32, tag="den")
                            nc.vector.tensor_scalar_max(den[:], psO[:, cout:cout + H], 1e-30)
                            dinv = epp.tile([P, H], F32, tag="dinv")
                            nc.vector.reciprocal(dinv[:], den[:])
                            hsb = epp.tile([P, cout], F32, tag="hsb")
                            hv = hsb[:].rearrange("p (h c) -> p h c", h=H)
                            pv = psO[:, 0:cout].rearrange("p (h c) -> p h c", h=H)
                            dv = dinv[:].rearrange("p (h o) -> p h o", o=1)
                            _, dvb = bass.broadcast_tensor_aps(pv, dv)
                            nc.vector.tensor_tensor(out=hv, in0=pv, in1=dvb, op=AL.mult)
                            hfin = epp.tile([P, cout], out_dt, tag="hfin")
                            nc.vector.tensor_tensor(out=hfin[:], in0=hsb[:], in1=biasV[:],
                                                    op=AL.add)
                            if relu:
                                nc.vector.tensor_scalar_max(hfin[:], hfin[:], 0.0)
                            nc.sync.dma_start(out=out_dram[s * P:(s + 1) * P, :],
                                              in_=hfin[:])

            # ---------- layer 1 ----------
            dense(lambda j: ap2d("xT")[:, j * P:(j + 1) * P],
                  C["W1l"], C["W1r"], C["blr1"], XLR1_loc, CO1, transpose_in=False)
            nc.gpsimd.collective_compute(
                "AllGather", AL.bypass, replica_groups=[list(range(NCORES))],
                ins=[XLR1_loc[:, :]], outs=[XLR1[:, :]])
            edge_phase(XLR1, CO1, H1, C["vV1"], C["attV1"], C["b1"], HL1, relu=True)
            # ---------- layer 2 ----------
            dense(lambda j: HL1[j * P:(j + 1) * P, :],
                  C["W2l"], C["W2r"], C["blr2"], XLR2_loc, CO2, transpose_in=True)
            nc.gpsimd.collective_compute(
                "AllGather", AL.bypass, replica_groups=[list(range(NCORES))],
                ins=[XLR2_loc[:, :]], outs=[XLR2[:, :]])
            edge_phase(XLR2, CO2, H2, C["vV2"], C["attV2"], C["b2"], out_d,
                       relu=False, out_dt=BF16)
    return nc, offs, NW


def _make_consts(W1_l, b1_l, W1_r, b1_r, W1_e, att1, bias1,
                 W2_l, b2_l, W2_r, b2_r, W2_e, att2, bias2):
    iota = np.repeat(np.arange(P, dtype=np.float32)[None, :], P, axis=0)
    return [
        iota,
        np.asarray(W1_l, np.float32), np.asarray(W1_r, np.float32),
        _rep(np.concatenate([np.asarray(b1_l).ravel(), np.asarray(b1_r).ravel()])),
        _rep(np.asarray(W1_e).ravel()), _rep(np.asarray(att1).ravel()),
        _rep(np.asarray(bias1).ravel()),
        np.asarray(W2_l, np.float32), np.asarray(W2_r, np.float32),
        _rep(np.concatenate([np.asarray(b2_l).ravel(), np.asarray(b2_r).ravel()])),
        _rep(np.asarray(W2_e).ravel()), _rep(np.asarray(att2).ravel()),
        _rep(np.asarray(bias2).ravel()),
    ]


def _warm_devices():
    """Establish the axon/PJRT session (network handshakes) while the main
    thread does CPU-bound prep/build; device_put releases the GIL."""
    try:
        import jax
        d = jax.devices()
        jax.device_put(np.zeros((8, 8), np.float32), d[0]).block_until_ready()
    except Exception:
        pass


def _fetch_parallel(arrays):
    """Convert possibly-device-resident (jax) arrays to numpy, overlapping
    the per-array transfers."""
    outs = [None] * len(arrays)

    def get(i):
        outs[i] = np.asarray(arrays[i])

    ths = [threading.Thread(target=get, args=(i,)) for i in range(len(arrays))]
    for t in ths:
        t.start()
    for t in ths:
        t.join()
    return outs


def kernel(x, edge_index, edge_attr,
           W1_l, b1_l, W1_r, b1_r, W1_e, att1, bias1,
           W2_l, b2_l, W2_r, b2_r, W2_e, att2, bias2):
    _mark("kernel start")
    warm = threading.Thread(target=_warm_devices, daemon=True)
    warm.start()
    if not isinstance(x, np.ndarray):
        x, edge_index, edge_attr = _fetch_parallel([x, edge_index, edge_attr])
        _mark("device inputs fetched")
    x = np.asarray(x, np.float32)
    prep = _prep_edges(edge_index, edge_attr)
    Tpad = prep["Tpad"]
    _mark("prep done")

    nc, offs, NW = _build_kernel(Tpad, prep["T_slot"])
    _mark("build done")

    consts = _make_consts(W1_l, b1_l, W1_r, b1_r, W1_e, att1, bias1,
                          W2_l, b2_l, W2_r, b2_r, W2_e, att2, bias2)
    cvec = np.concatenate([c.ravel() for c in consts]).astype(np.float32)

    xpad = np.zeros((NTOT, F_IN), np.float32)
    xpad[:N_NODES] = x
    xT = np.ascontiguousarray(xpad.T)          # [128, NTOT]

    in_maps = []
    for k in range(NCORES):
        blobv = np.empty((1, NW), np.float32)
        o = 0
        blobv[0, o:o + cvec.size] = cvec
        o += cvec.size
        blobv[0, o:o + P * NLOC] = xT[:, k * NLOC:(k + 1) * NLOC].ravel()
        o += P * NLOC
        for arr in (prep["idx_st"][k].view(np.float32),
                    prep["dst_st"][k].view(np.float32),
                    prep["dr_st"][k], prep["ea_st"][k]):
            blobv[0, o:o + P * Tpad] = arr.ravel()
            o += P * Tpad
        assert o == NW
        in_maps.append({"blob": blobv})
    _mark("blobs packed")
    warm.join(timeout=120)

    res = run_bass_kernel_spmd(nc, in_maps, core_ids=list(range(NCORES)))
    _mark("spmd run done")
    out = np.concatenate([res.results[k]["out"] for k in range(NCORES)], axis=0)
    _mark("fetch done")
    return out[:N_NODES].astype(np.float32)


# revision 48
# speedup vs baseline: 2.2539x; 1.1094x over previous
"""GATv2 (2-layer) Trainium2 Bass kernel, 8-core SPMD, single fused NEFF.

Wall-clock-oriented design (device exec is ~0.1s; build/compile/transfer
dominate):
- ONE kernel for both layers; h is exchanged on-device with an AllGather
  collective (no inter-layer host round trip, one compile, one launch).
- Uniform node sharding: 784 blocks of 128 nodes, 98 blocks per core, so
  AllGather slices concatenate into global node order and one edge-index
  stream serves both layers.
- xl/xr tables are row-interleaved ([2N, C]: row 2n = xl_n, 2n+1 = xr_n),
  so gathers use indices 2*src and 2*dst+1 into the same table.
- All per-core inputs are packed into a single f32 blob (one sharded
  transfer); int32 stream regions are viewed via AP.bitcast.
- Edge phase: per 128-edge tile only 5 instructions (2 gathers, one-hot
  build, exp-prescale into an rhs buffer that also carries the exp column,
  and ONE aggregation matmul over [cout+H] columns); the logit pipeline is
  batched over CH=32 tiles with broadcast APs.
- Segment softmax without max subtraction (logits are O(1); exact enough),
  denominator applied after aggregation. leaky_relu via 0.6x + 0.4|x|.
- Final output in bf16 (value-proportional rounding keeps relative error
  safe); inputs/tables stay f32 (bf16 there creates absolute-scale errors
  that blow up the relative-error metric at near-zero outputs).
"""

import json
import os
import sys
import threading
import time as _time
import numpy as np

# Smaller/faster NEFF packaging (no debug info); read by walrus arg builder.
os.environ.setdefault("CONCOURSE_SCRUB_NEFF_DEBUG_INFO", "1")

_T0 = _time.time()


def _mark(msg):
    print(f"[kernel +{_time.time() - _T0:6.2f}s] {msg}", file=sys.stderr, flush=True)

import concourse.bass as bass
import concourse.mybir as mybir
from concourse.tile import TileContext, ScopedClock
from concourse.bass_utils import run_bass_kernel_spmd
from concourse.masks import make_identity

# ----------------------------------------------------------------------------
# Workarounds for the walrus build in this container: at most ONE sync-wait
# per instruction. Extra waits are peeled onto NoOps inserted just before.
# ----------------------------------------------------------------------------
_MAXW = 1
_split_counter = [0]


def _patched_drain_and_barrier(self, tick_clock, wait_clock):
    d0 = self.nc.sync.drain()
    wait_clock.add_sem_waits(d0.ins, ScopedClock({None: tick_clock.global_clock}))
    waits = list(d0.ins.sync_info.on_wait)
    if len(waits) > _MAXW:
        del d0.ins.sync_info.on_wait[_MAXW:]
        rest = waits[_MAXW:]
        for i in range(0, len(rest), _MAXW):
            d = self.nc.sync.drain()
            if d.ins.sync_info is None:
                d.ins.sync_info = mybir.SyncInfo(on_update=[], on_wait=[])
            d.ins.sync_info.on_wait.extend(rest[i:i + _MAXW])
    self.nc.all_engine_barrier()
    popped = self.nc._tile_sem_poison_stack.pop()
    assert popped is self._sem_poison
    self.nc.clear_and_free_semaphores(list(self.sems.allocated().values()))
    self.nc.all_engine_barrier()


def _fix_bir_json(data: bytes) -> bytes:
    try:
        import orjson
        _loads, _dumps = orjson.loads, lambda m: orjson.dumps(m)
    except ImportError:
        _loads, _dumps = json.loads, lambda m: json.dumps(m).encode()
    m = _loads(data)
    changed = False
    for f in m.get("functions", []):
        for b in f.get("blocks", []):
            insts = b.get("instructions")
            if not insts:
                continue
            out = []
            for inst in insts:
                si = inst.get("sync_info") or {}
                waits = si.get("on_wait") or []
                if len(waits) > 1:
                    for w in waits[:-1]:
                        _split_counter[0] += 1
                        out.append({
                            "name": f"I-sw{_split_counter[0]}",
                            "opcode": "NoOp",
                            "engine": inst.get("engine"),
                            "ins": [], "outs": [],
                            "sync_info": {"on_update": [], "on_wait": [w]},
                        })
                    si["on_wait"] = [waits[-1]]
                    changed = True
                out.append(inst)
            b["instructions"] = out
    if not changed:
        return data
    return _dumps(m)


def _install_fixes():
    TileContext._drain_and_barrier = _patched_drain_and_barrier
    if not getattr(bass.Bass, "_tilefix_json", False):
        orig = bass.Bass.to_json_bytes

        def to_json_bytes(self, *a, **k):
            return _fix_bir_json(orig(self, *a, **k))

        bass.Bass.to_json_bytes = to_json_bytes
        bass.Bass._tilefix_json = True


_install_fixes()


def _install_fast_walrus():
    """Skip the birverifier pass (validation-only; this BIR is known-valid)
    to cut client-side compile time."""
    import concourse.bass_utils as bu
    from pathlib import Path
    from concourse.aot_env import aot_getenv

    if getattr(bu, "_fast_walrus", False):
        return

    def fast_bvo(tmpdir, inp="bir.json", outp="file.neff", arch=None, *,
                 dve_root=None):
        cmd = [
            bu.get_walrus_driver(),
            "--pass",
            ",".join(["runtime_memory_reservation", "lower_act", "lower_dve",
                      "lower_ap_offset", "codegen", "neff_packager"]),
            "-i", inp,
            "--neff-output-filename", outp,
            "--enable-birsim=true",
            "--mem-mode=physical",
            "--policy=0",
            "--enable-ldw-opt=false",
            "--assign-static-dmas-to-sp=false",
            f"--dram-page-size={aot_getenv('NEURON_SCRATCHPAD_PAGE_SIZE', '256')}",
            "--enable-neff-debug-info=false",
            "--jobs", "8",
            *bu.get_walrus_args(
                bu.get_bir_arch(tmpdir, inp) if arch is None else arch,
                tmpdir, dve_root=dve_root),
        ]
        result = bu.run_command(cmd, cwd=tmpdir)
        if result is not None:
            (Path(tmpdir) / "log.txt").write_text(result.stdout)
        return f"{tmpdir}/{outp}"

    bu.bir_verify_and_optimise = fast_bvo
    bu._fast_walrus = True


_install_fast_walrus()

# ----------------------------------------------------------------------------
N_NODES = 100_000
N_EDGES = 1_600_000
F_IN = 128
H1, C1 = 2, 64
H2, C2 = 1, 64
CO1, CO2 = H1 * C1, H2 * C2            # 128, 64
NCORES = 8
P = 128
NBLKC = 98                              # blocks per core
NLOC = NBLKC * P                        # 12544 nodes per core
NTOT = NCORES * NLOC                    # 100352 padded nodes
CH = 32                                 # tiles per merged logit chunk
F32 = mybir.dt.float32
BF16 = mybir.dt.bfloat16
I32 = mybir.dt.int32
AL = mybir.AluOpType
AF = mybir.ActivationFunctionType


def _rep(v):
    v = np.asarray(v, np.float32).reshape(1, -1)
    return np.ascontiguousarray(np.repeat(v, P, axis=0))


def _prep_edges(edge_index, edge_attr):
    """Sort edges by dst; build per-core [128, Tpad] streams (vectorized)."""
    src = np.asarray(edge_index[0], np.int64)
    dst = np.asarray(edge_index[1], np.int64)
    E = src.shape[0]
    order = np.argsort(dst, kind="stable")
    src_s = src[order].astype(np.int64)
    dst_s = dst[order].astype(np.int64)
    ea_s = np.asarray(edge_attr, np.float32).reshape(-1)[order]
    blk = (dst_s >> 7).astype(np.int64)            # global block 0..781
    cnt = np.bincount(blk, minlength=NCORES * NBLKC)
    T_slot = np.maximum((cnt.reshape(NCORES, NBLKC) + P - 1) // P, 1).max(axis=0)
    col0 = np.zeros(NBLKC + 1, np.int64)
    col0[1:] = np.cumsum(T_slot)
    sumT = int(col0[-1])
    Tpad = ((sumT + CH - 1) // CH) * CH
    T_slot = T_slot.copy()
    T_slot[-1] += Tpad - sumT                      # tail pad columns absorb
    runstart = np.zeros(NCORES * NBLKC + 1, np.int64)
    runstart[1:] = np.cumsum(cnt)
    rank = np.arange(E, dtype=np.int64) - runstart[blk]
    core = blk // NBLKC
    slot = blk - core * NBLKC
    col = col0[slot] + (rank >> 7)
    row = rank & 127

    idx_st = np.zeros((NCORES, P, Tpad), np.int32)
    dst_st = np.ones((NCORES, P, Tpad), np.int32)
    dr_st = np.full((NCORES, P, Tpad), -1.0, np.float32)
    ea_st = np.zeros((NCORES, P, Tpad), np.float32)
    idx_st[core, row, col] = (2 * src_s).astype(np.int32)
    dst_st[core, row, col] = (2 * dst_s + 1).astype(np.int32)
    dr_st[core, row, col] = (dst_s & 127).astype(np.float32)
    ea_st[core, row, col] = ea_s
    return dict(Tpad=Tpad, T_slot=T_slot.astype(np.int64), idx_st=idx_st,
                dst_st=dst_st, dr_st=dr_st, ea_st=ea_st)


def _build_kernel(Tpad, T_slot):
    nc = bass.Bass()

    # ---- blob layout (element offsets into the per-core [1, NW] f32 blob)
    widths = dict(iotaV=P, W1l=CO1, W1r=CO1, blr1=2 * CO1, vV1=CO1,
                  attV1=CO1, b1=CO1, W2l=CO2, W2r=CO2, blr2=2 * CO2,
                  vV2=CO2, attV2=CO2, b2=CO2)
    offs = {}
    off = 0
    for k, w in widths.items():
        offs[k] = off
        off += P * w
    offs["xT"] = off
    off += P * NLOC
    for s in ("idx", "dstg", "dr", "ea"):
        offs[s] = off
        off += P * Tpad
    NW = off

    blob = nc.dram_tensor("blob", [1, NW], F32, kind="ExternalInput")
    out_d = nc.dram_tensor("out", [NLOC, CO2], BF16, kind="ExternalOutput")
    XLR1_loc = nc.dram_tensor("XLR1_loc", [2 * NLOC, CO1], F32)
    XLR1 = nc.dram_tensor("XLR1", [2 * NTOT, CO1], F32)
    HL1 = nc.dram_tensor("HL1", [NLOC, CO1], F32)
    XLR2_loc = nc.dram_tensor("XLR2_loc", [2 * NLOC, CO2], F32)
    XLR2 = nc.dram_tensor("XLR2", [2 * NTOT, CO2], F32)

    def ap2d(name, w=None):
        o, tw = offs[name], widths.get(name, Tpad if name in
                                       ("idx", "dstg", "dr", "ea") else None)
        if name == "xT":
            tw = NLOC
        if w is None:
            w = tw
        return blob[0:1, o:o + P * tw].rearrange("o (p w) -> (o p) w", p=P)

    with TileContext(nc) as tc:
        with (
            tc.tile_pool(name="const", bufs=1) as cp,
            tc.tile_pool(name="dense", bufs=3) as dp,
            tc.tile_pool(name="st", bufs=3) as sp,
            tc.tile_pool(name="chunk", bufs=2) as chp,
            tc.tile_pool(name="tile", bufs=6) as tp,
            tc.tile_pool(name="ep", bufs=2) as epp,
            tc.tile_pool(name="pd", bufs=2, space="PSUM") as ppd,
            tc.tile_pool(name="po", bufs=2, space="PSUM") as ppo,
            tc.tile_pool(name="pt", bufs=2, space="PSUM") as ppt,
        ):
            C = {}
            for k, w in widths.items():
                t = cp.tile([P, w], F32, tag=f"c_{k}")
                nc.sync.dma_start(out=t[:], in_=ap2d(k))
                C[k] = t
            ident = cp.tile([P, P], F32)
            make_identity(nc, ident[:])
            Szero = cp.tile([P, P], F32)
            nc.vector.tensor_scalar(out=Szero[:], in0=ident[:], scalar1=0.0,
                                    scalar2=None, op0=AL.mult)

            def dense(xsrc_ap_of_blk, Wl, Wr, blr, dst_dram, cout, transpose_in):
                for j in range(NBLKC):
                    if transpose_in:
                        ht = dp.tile([P, P], F32, tag="ht")
                        nc.sync.dma_start(out=ht[:], in_=xsrc_ap_of_blk(j))
                        pT = ppt.tile([P, P], F32, space="PSUM")
                        nc.tensor.transpose(out=pT[:], in_=ht[:],
                                            identity=ident[:])
                        xt = dp.tile([P, P], F32, tag="xt")
                        nc.scalar.copy(xt[:], pT[:])
                    else:
                        xt = dp.tile([P, P], F32, tag="xt")
                        nc.sync.dma_start(out=xt[:], in_=xsrc_ap_of_blk(j))
                    ps = ppd.tile([P, 2 * cout], F32, space="PSUM")
                    nc.tensor.matmul(ps[:, 0:cout], lhsT=xt[:], rhs=Wl[:],
                                     start=True, stop=True)
                    nc.tensor.matmul(ps[:, cout:2 * cout], lhsT=xt[:], rhs=Wr[:],
                                     start=True, stop=True)
                    xlr = dp.tile([P, 2 * cout], F32, tag="xlr")
                    nc.vector.tensor_tensor(out=xlr[:], in0=ps[:], in1=blr[:],
                                            op=AL.add)
                    oap = dst_dram[j * 2 * P:(j + 1) * 2 * P, :].rearrange(
                        "(p two) c -> p (two c)", two=2)
                    nc.sync.dma_start(out=oap, in_=xlr[:])

            def edge_phase(table, cout, H, vV, attV, biasV, out_dram, relu,
                           out_dt=F32):
                Cc = cout // H
                # block bookkeeping per global column
                blk_of, start_c, stop_c = [], [], []
                for s in range(NBLKC):
                    for t in range(int(T_slot[s])):
                        blk_of.append(s)
                        start_c.append(t == 0)
                        stop_c.append(t == int(T_slot[s]) - 1)
                psO = None
                for g in range(Tpad // CH):
                    idxc = sp.tile([P, CH], I32, tag="idxc")
                    nc.sync.dma_start(out=idxc[:], in_=ap2d("idx")[:, g * CH:(g + 1) * CH].bitcast(I32))
                    dstc = sp.tile([P, CH], I32, tag="dstc")
                    nc.sync.dma_start(out=dstc[:], in_=ap2d("dstg")[:, g * CH:(g + 1) * CH].bitcast(I32))
                    drc = sp.tile([P, CH], F32, tag="drc")
                    nc.sync.dma_start(out=drc[:], in_=ap2d("dr")[:, g * CH:(g + 1) * CH])
                    eac = sp.tile([P, CH], F32, tag="eac")
                    nc.sync.dma_start(out=eac[:], in_=ap2d("ea")[:, g * CH:(g + 1) * CH])

                    W = cout + H          # rhs row: [scaled msg | ex] (or
                    #                       [raw msg | ones] when H == 1)
                    # allocate at layer-1 sizes so L2 reuses the same slots;
                    # only the leading columns are used.
                    msgA_t = chp.tile([P, CH * CO1], F32, tag="msgA")
                    m_t = chp.tile([P, CH * CO1], F32, tag="m")
                    wk_t = chp.tile([P, CH * CO1], F32, tag="wk")
                    tabs_t = chp.tile([P, CH * CO1], F32, tag="tabs")
                    m = m_t[:, 0:CH * cout]
                    wk = wk_t[:, 0:CH * cout]
                    tabs = tabs_t[:, 0:CH * cout]
                    if H == 1:
                        # H==1 fast path: gathers land in rhs layout directly
                        # (stride W per tile) with a ones column at [cout];
                        # the one-hot gets pre-scaled by exp instead.
                        msgA3 = msgA_t[:, 0:CH * W].rearrange(
                            "p (t w) -> p t w", w=W)[:, :, 0:cout]
                        rhs = None
                    else:
                        rhs_t = chp.tile([P, CH * (CO1 + H1)], F32, tag="rhs")
                        rhs = rhs_t[:, 0:CH * W]
                        msgA = msgA_t[:, 0:CH * cout]
                        msgA3 = msgA[:].rearrange("p (t c) -> p t c", t=CH)
                    stride = W if H == 1 else cout
                    for t in range(CH):
                        nc.gpsimd.indirect_dma_start(
                            out=msgA_t[:, t * stride:t * stride + cout],
                            out_offset=None, in_=table[:, :],
                            in_offset=bass.IndirectOffsetOnAxis(ap=idxc[:, t:t + 1], axis=0))
                        nc.gpsimd.indirect_dma_start(
                            out=m[:, t * cout:(t + 1) * cout], out_offset=None,
                            in_=table[:, :],
                            in_offset=bass.IndirectOffsetOnAxis(ap=dstc[:, t:t + 1], axis=0))
                    if H == 1:
                        # ones column per tile slot (denominator via matmul)
                        onescols = msgA_t[:, 0:CH * W].rearrange(
                            "p (t w) -> p t w", w=W)[:, :, cout:cout + 1]
                        src1 = C["iotaV"][:, 0:CH].rearrange("p (t o) -> p t o", o=1)
                        nc.vector.tensor_scalar(out=onescols, in0=src1,
                                                scalar1=0.0, scalar2=1.0,
                                                op0=AL.mult, op1=AL.add)
                    # m = msgA + xr[dst] ; m += ea * vV (broadcast)
                    mv = m[:].rearrange("p (t c) -> p t c", t=CH)
                    nc.vector.tensor_tensor(out=mv, in0=mv, in1=msgA3, op=AL.add)
                    eb = eac[:].rearrange("p (t o) -> p t o", o=1)
                    vb = vV[:].rearrange("p (o c) -> p o c", o=1)
                    ebb, vbb = bass.broadcast_tensor_aps(eb, vb)
                    wkv = wk[:].rearrange("p (t c) -> p t c", t=CH)
                    nc.vector.tensor_tensor(out=wkv, in0=ebb, in1=vbb, op=AL.mult)
                    nc.vector.tensor_tensor(out=m[:], in0=m[:], in1=wk[:], op=AL.add)
                    # tabs = |m| ; q = m*att ; lin = reduce ; u = |m|*att ; ur
                    nc.scalar.activation(tabs[:], m[:], AF.Abs)
                    av = attV[:].rearrange("p (o c) -> p o c", o=1)
                    _, avb = bass.broadcast_tensor_aps(mv, av)
                    nc.vector.tensor_tensor(out=wkv, in0=mv, in1=avb, op=AL.mult)
                    lin = sp.tile([P, CH * H], F32, tag="lin")
                    nc.vector.tensor_reduce(out=lin[:],
                                            in_=wk[:].rearrange("p (th c) -> p th c", c=Cc),
                                            axis=mybir.AxisListType.X, op=AL.add)
                    tv = tabs[:].rearrange("p (t c) -> p t c", t=CH)
                    nc.vector.tensor_tensor(out=wkv, in0=tv, in1=avb, op=AL.mult)
                    ur = sp.tile([P, CH * H], F32, tag="ur")
                    nc.vector.tensor_reduce(out=ur[:],
                                            in_=wk[:].rearrange("p (th c) -> p th c", c=Cc),
                                            axis=mybir.AxisListType.X, op=AL.add)
                    logit = sp.tile([P, CH * H], F32, tag="logit")
                    nc.vector.tensor_scalar(out=logit[:], in0=lin[:], scalar1=0.6,
                                            scalar2=None, op0=AL.mult)
                    nc.vector.scalar_tensor_tensor(out=logit[:], in0=ur[:], scalar=0.4,
                                                   in1=logit[:], op0=AL.mult, op1=AL.add)
                    ex = sp.tile([P, CH * H], F32, tag="ex")
                    nc.scalar.activation(ex[:], logit[:], AF.Exp)
                    if H > 1:
                        # copy ex into the tail H columns of each rhs slot
                        exdst = rhs[:].rearrange("p (t w) -> p t w", w=W)[:, :, cout:cout + H]
                        nc.scalar.copy(exdst, ex[:].rearrange("p (t h) -> p t h", h=H))

                    for t in range(CH):
                        c = g * CH + t
                        s = blk_of[c]
                        if start_c[c]:
                            psO = ppo.tile([P, W], F32, space="PSUM")
                            # the first start=True accumulation is dropped by
                            # HW; absorb it with a zero matmul per block.
                            nc.tensor.matmul(psO[:], lhsT=Szero[:],
                                             rhs=C["blr1"][:, 0:W],
                                             start=True, stop=False)
                        S01 = tp.tile([P, P], F32, tag="S01")
                        if H == 1:
                            # one-hot pre-scaled by exp; rhs = [raw msg | 1]
                            nc.vector.tensor_scalar(out=S01[:], in0=C["iotaV"][:],
                                                    scalar1=drc[:, t:t + 1],
                                                    scalar2=ex[:, t:t + 1],
                                                    op0=AL.is_equal, op1=AL.mult)
                            rhs_slice = msgA_t[:, t * W:(t + 1) * W]
                        else:
                            nc.vector.tensor_scalar(out=S01[:], in0=C["iotaV"][:],
                                                    scalar1=drc[:, t:t + 1],
                                                    scalar2=None, op0=AL.is_equal)
                            # scaled = msgA_tile * ex (per-head) -> rhs slot
                            sc = rhs[:, t * W:t * W + cout].rearrange(
                                "p (h c) -> p h c", h=H)
                            mg = msgA[:, t * cout:(t + 1) * cout].rearrange(
                                "p (h c) -> p h c", h=H)
                            eview = ex[:, t * H:(t + 1) * H].rearrange(
                                "p (h o) -> p h o", o=1)
                            _, evb = bass.broadcast_tensor_aps(mg, eview)
                            nc.vector.tensor_tensor(out=sc, in0=mg, in1=evb,
                                                    op=AL.mult)
                            rhs_slice = rhs[:, t * W:(t + 1) * W]
                        nc.tensor.matmul(psO[:], lhsT=S01[:], rhs=rhs_slice,
                                         start=False, stop=bool(stop_c[c]))
                        if stop_c[c]:
                            den = epp.tile([P, H], F32, tag="den")
                            nc.vector.tensor_scalar_max(den[:], psO[:, cout:cout + H], 1e-30)
                            dinv = epp.tile([P, H], F32, tag="dinv")
                            nc.vector.reciprocal(dinv[:], den[:])
                            hsb = epp.tile([P, cout], F32, tag="hsb")
                            hv = hsb[:].rearrange("p (h c) -> p h c", h=H)
                            pv = psO[:, 0:cout].rearrange("p (h c) -> p h c", h=H)
                            dv = dinv[:].rearrange("p (h o) -> p h o", o=1)
                            _, dvb = bass.broadcast_tensor_aps(pv, dv)
                            nc.vector.tensor_tensor(out=hv, in0=pv, in1=dvb, op=AL.mult)
                            hfin = epp.tile([P, cout], out_dt, tag="hfin")
                            nc.vector.tensor_tensor(out=hfin[:], in0=hsb[:], in1=biasV[:],
                                                    op=AL.add)
                            if relu:
                                nc.vector.tensor_scalar_max(hfin[:], hfin[:], 0.0)
                            nc.sync.dma_start(out=out_dram[s * P:(s + 1) * P, :],
                                              in_=hfin[:])

            # ---------- layer 1 ----------
            dense(lambda j: ap2d("xT")[:, j * P:(j + 1) * P],
                  C["W1l"], C["W1r"], C["blr1"], XLR1_loc, CO1, transpose_in=False)
            nc.gpsimd.collective_compute(
                "AllGather", AL.bypass, replica_groups=[list(range(NCORES))],
                ins=[XLR1_loc[:, :]], outs=[XLR1[:, :]])
            edge_phase(XLR1, CO1, H1, C["vV1"], C["attV1"], C["b1"], HL1, relu=True)
            # ---------- layer 2 ----------
            dense(lambda j: HL1[j * P:(j + 1) * P, :],
                  C["W2l"], C["W2r"], C["blr2"], XLR2_loc, CO2, transpose_in=True)
            nc.gpsimd.collective_compute(
                "AllGather", AL.bypass, replica_groups=[list(range(NCORES))],
                ins=[XLR2_loc[:, :]], outs=[XLR2[:, :]])
            edge_phase(XLR2, CO2, H2, C["vV2"], C["attV2"], C["b2"], out_d,
                       relu=False, out_dt=BF16)
    return nc, offs, NW


def _make_consts(W1_l, b1_l, W1_r, b1_r, W1_e, att1, bias1,
                 W2_l, b2_l, W2_r, b2_r, W2_e, att2, bias2):
    iota = np.repeat(np.arange(P, dtype=np.float32)[None, :], P, axis=0)
    return [
        iota,
        np.asarray(W1_l, np.float32), np.asarray(W1_r, np.float32),
        _rep(np.concatenate([np.asarray(b1_l).ravel(), np.asarray(b1_r).ravel()])),
        _rep(np.asarray(W1_e).ravel()), _rep(np.asarray(att1).ravel()),
        _rep(np.asarray(bias1).ravel()),
        np.asarray(W2_l, np.float32), np.asarray(W2_r, np.float32),
        _rep(np.concatenate([np.asarray(b2_l).ravel(), np.asarray(b2_r).ravel()])),
        _rep(np.asarray(W2_e).ravel()), _rep(np.asarray(att2).ravel()),
        _rep(np.asarray(bias2).ravel()),
    ]


def _warm_devices():
    """Establish the axon/PJRT session (network handshakes) while the main
    thread does CPU-bound prep/build; device_put releases the GIL."""
    try:
        import jax
        d = jax.devices()
        jax.device_put(np.zeros((8, 8), np.float32), d[0]).block_until_ready()
    except Exception:
        pass


def _fetch_parallel(arrays):
    """Convert possibly-device-resident (jax) arrays to numpy, overlapping
    the per-array transfers."""
    outs = [None] * len(arrays)

    def get(i):
        outs[i] = np.asarray(arrays[i])

    ths = [threading.Thread(target=get, args=(i,)) for i in range(len(arrays))]
    for t in ths:
        t.start()
    for t in ths:
        t.join()
    return outs


def kernel(x, edge_index, edge_attr,
           W1_l, b1_l, W1_r, b1_r, W1_e, att1, bias1,
           W2_l, b2_l, W2_r, b2_r, W2_e, att2, bias2):
    _mark("kernel start")
    warm = threading.Thread(target=_warm_devices, daemon=True)
    warm.start()
    if not isinstance(x, np.ndarray):
        x, edge_index, edge_attr = _fetch_parallel([x, edge_index, edge_attr])
        _mark("device inputs fetched")
    x = np.asarray(x, np.float32)
    prep = _prep_edges(edge_index, edge_attr)
    Tpad = prep["Tpad"]
    _mark("prep done")

    nc, offs, NW = _build_kernel(Tpad, prep["T_slot"])
    _mark("build done")

    consts = _make_consts(W1_l, b1_l, W1_r, b1_r, W1_e, att1, bias1,
                          W2_l, b2_l, W2_r, b2_r, W2_e, att2, bias2)
    cvec = np.concatenate([c.ravel() for c in consts]).astype(np.float32)

    xpad = np.zeros((NTOT, F_IN), np.float32)
    xpad[:N_NODES] = x
    xT = np.ascontiguousarray(xpad.T)          # [128, NTOT]

    in_maps = []
    for k in range(NCORES):
        blobv = np.empty((1, NW), np.float32)
        o = 0
        blobv[0, o:o + cvec.size] = cvec
        o += cvec.size
        blobv[0, o:o + P * NLOC] = xT[:, k * NLOC:(k + 1) * NLOC].ravel()
        o += P * NLOC
        for arr in (prep["idx_st"][k].view(np.float32),
                    prep["dst_st"][k].view(np.float32),
                    prep["dr_st"][k], prep["ea_st"][k]):
            blobv[0, o:o + P * Tpad] = arr.ravel()
            o += P * Tpad
        assert o == NW
        in_maps.append({"blob": blobv})
    _mark("blobs packed")
    warm.join(timeout=120)

    res = run_bass_kernel_spmd(nc, in_maps, core_ids=list(range(NCORES)))
    _mark("spmd run done")
    out = np.concatenate([res.results[k]["out"] for k in range(NCORES)], axis=0)
    _mark("fetch done")
    return out[:N_NODES].astype(np.float32)
